# revision 20
# baseline (speedup 1.0000x reference)
"""Multi-head attention (B=1, S=4096, D=768, H=12) on 8 trn2 NeuronCores.

Sharding: 4 query shards x 2 head groups (6 heads each). Each core:
  - projects its 1024 q rows to QT [384, 1024] for its 6 heads
  - projects the full k, v to KT [384, 4096] / V_aug [4096, 6, 65]
    for its 6 heads (the 65th V column is ones so the probs@V matmul
    also produces the softmax denominator)
  - scores sT = K_h @ Q_h^T per head/k-tile on PE (K=64 contraction;
    the two heads of a pair sit on PE row-groups 0-63 / 64-127 so their
    score matmuls run concurrently)
  - exp on ACT directly from PSUM (scale=1/8, bias=-3 folded in; the -3
    is softmax-invariant and buys fp16 range), probs fp16 to SBUF
  - mask applied MULTIPLICATIVELY to probs on DVE (fp16 tensor_tensor
    at 2x rate) -- no fp32 PSUM pre-pass, which saturated DVE in v1
  - probs@V_aug accumulated on PE, normalization at the end
  - partial output projection with its 6 heads' wo slice; host adds the
    two head-group partials.  bv/bo are folded into a host-computed
    output bias (bv contribution = den * bv^T, and den normalizes to 1,
    so bo' = bv_hg @ wo_hg + bo).  bk is applied in the K projection.

Inputs are pre-transposed/pre-tiled/cast to fp16 on the host (layout
prep is part of sharding); all matmul accumulation is fp32 in PSUM.
"""

import numpy as np

import concourse.bass as bass
import concourse.mybir as mybir
import concourse.tile as tile
from concourse import bacc, bass_utils

B, S, D, H = 1, 4096, 768, 12
DK = D // H  # 64
NCORES = 8
NQSH = 4  # query shards
NHG = 2  # head groups
SQ = S // NQSH  # 1024 query rows per core
HL = H // NHG  # 6 local heads
NPAIR = HL // 2  # 3 local head pairs
CH = HL * DK  # 384 local channels
DT_IN = D // 128  # 6 input-dim tiles
DT_CH = CH // 128  # 3 channel tiles
KT_TILES = S // 128  # 32 k tiles
NCH = S // 512  # 8 column chunks for full-seq K/V projections
QC = SQ // 512  # 2 query column chunks

F16 = mybir.dt.float16
F32 = mybir.dt.float32

_CACHE = {}


def build_kernel(timing=False):
    nc = bacc.Bacc("TRN2", target_bir_lowering=False, debug=False, num_devices=NCORES)

    # timing=True: declare the big inputs as Internal DRAM (garbage contents,
    # identical DMA traffic) so per-exec host->device input copies don't
    # dominate the wall-clock slope measurement.
    kw = {} if timing else {"kind": "ExternalInput"}
    qT = nc.dram_tensor("qT", [128, DT_IN, SQ], F16, **kw)
    kT = nc.dram_tensor("kT", [128, DT_IN, S], F16, **kw)
    vT = nc.dram_tensor("vT", [128, DT_IN, S], F16, **kw)
    maskT = nc.dram_tensor("maskT", [128, KT_TILES, SQ], F16, **kw)
    wq = nc.dram_tensor("wq", [128, DT_IN, CH], F16, **kw)
    wk = nc.dram_tensor("wk", [128, DT_IN, CH], F16, **kw)
    wv = nc.dram_tensor("wv", [128, DT_IN, CH], F16, **kw)
    wo = nc.dram_tensor("wo", [128, DT_CH, D], F16, **kw)
    bq = nc.dram_tensor("bq", [128, DT_CH], F32, **kw)
    bk = nc.dram_tensor("bk", [128, DT_CH], F32, **kw)
    bo = nc.dram_tensor("bo", [128, DT_IN], F32, **kw)
    if timing:
        nc.dram_tensor("tinput", [1, 8], F32, kind="ExternalInput")
    outT = nc.dram_tensor("outT", [D, SQ], F16, kind="ExternalOutput")

    with tile.TileContext(nc) as tc:
        _build_tile(tc, qT, kT, vT, maskT, wq, wk, wv, wo, bq, bk, bo, outT)
    nc.compile()
    return nc


def _build_tile(tc, qT, kT, vT, maskT, wq, wk, wv, wo, bq, bk, bo, outT):
    nc = tc.nc

    with (
        tc.tile_pool(name="persist", bufs=1) as persist,
        tc.tile_pool(name="stage", bufs=3) as stage,
        tc.tile_pool(name="probs", bufs=2) as probs_pool,
        tc.tile_pool(name="small", bufs=1) as small,
        tc.tile_pool(name="osb", bufs=2) as osb_pool,
    ):
        # ---- persistent SBUF tensors ----
        wq_sb = persist.tile([128, DT_IN, CH], F16, name="wq_sb")
        nc.sync.dma_start(out=wq_sb[:], in_=wq[:])
        q_in = persist.tile([128, DT_IN, SQ], F16, name="q_in")
        nc.sync.dma_start(out=q_in[:], in_=qT[:])
        wk_sb = persist.tile([128, DT_IN, CH], F16, name="wk_sb")
        nc.sync.dma_start(out=wk_sb[:], in_=wk[:])
        wv_sb = persist.tile([128, DT_IN, CH], F16, name="wv_sb")
        nc.sync.dma_start(out=wv_sb[:], in_=wv[:])

        bq_sb = persist.tile([128, DT_CH], F32, name="bq_sb")
        nc.sync.dma_start(out=bq_sb[:], in_=bq[:])
        bk_sb = persist.tile([128, DT_CH], F32, name="bk_sb")
        nc.sync.dma_start(out=bk_sb[:], in_=bk[:])
        bo_sb = persist.tile([128, DT_IN], F32, name="bo_sb")
        nc.sync.dma_start(out=bo_sb[:], in_=bo[:])

        # wo is only needed for the output projection at the very end; issue
        # it after the input stream so it doesn't delay kT/vT/mask chunks.
        wo_sb = persist.tile([128, DT_CH, D], F16, name="wo_sb")

        # mask chunks are DMA'd just-in-time interleaved with the vT chunks
        # (below) so attention's first k-tiles aren't gated behind one big
        # 8.4MB transfer queued ahead of the kT/vT stream.
        maskT_sb = persist.tile([128, KT_TILES, SQ], F16, name="maskT_sb")

        KT_sb = persist.tile([128, DT_CH, S], F16, name="KT_sb")
        V_sb = persist.tile([128, KT_TILES, HL, DK + 1], F16, name="V_sb")
        QT_sb = persist.tile([128, DT_CH, SQ], F16, name="QT_sb")
        ctx_sb = persist.tile([128, DT_CH, SQ], F16, name="ctx_sb")

        # ones column for the V augmentation (denominator trick)
        nc.vector.memset(V_sb[:, :, :, DK : DK + 1], 1.0)

        # per-partition bias AP for exp's softmax-invariant -3 shift
        expbias = persist.tile([128, 1], F32, name="expbias")
        nc.vector.memset(expbias[:], -3.0)

        # ---- projections ----
        with (
            tc.tile_pool(name="pj", bufs=2, space="PSUM") as pj_pool,
            tc.tile_pool(name="pvp", bufs=2, space="PSUM") as pv_pool,
        ):
            # Q projection -> QT_sb [chan, q]
            for ch in range(DT_CH):
                for qc in range(QC):
                    ps = pj_pool.tile([128, 512], F32, tag="pj")
                    for ka in range(DT_IN):
                        nc.tensor.matmul(
                            ps[:],
                            wq_sb[:, ka, ch * 128 : (ch + 1) * 128],
                            q_in[:, ka, qc * 512 : (qc + 1) * 512],
                            start=(ka == 0),
                            stop=(ka == DT_IN - 1),
                        )
                    nc.vector.tensor_scalar_add(
                        out=QT_sb[:, ch, qc * 512 : (qc + 1) * 512],
                        in0=ps[:],
                        scalar1=bq_sb[:, ch : ch + 1],
                    )

            # K projection -> KT_sb [chan, seq]
            for nch in range(NCH):
                x_sb = stage.tile([128, DT_IN, 512], F16, tag="xT")
                nc.sync.dma_start(out=x_sb[:], in_=kT[:, :, nch * 512 : (nch + 1) * 512])
                for ch in range(DT_CH):
                    ps = pj_pool.tile([128, 512], F32, tag="pj")
                    for ka in range(DT_IN):
                        nc.tensor.matmul(
                            ps[:],
                            wk_sb[:, ka, ch * 128 : (ch + 1) * 128],
                            x_sb[:, ka, :],
                            start=(ka == 0),
                            stop=(ka == DT_IN - 1),
                        )
                    nc.vector.tensor_scalar_add(
                        out=KT_sb[:, ch, nch * 512 : (nch + 1) * 512],
                        in0=ps[:],
                        scalar1=bk_sb[:, ch : ch + 1],
                    )

            # V projection -> V_sb [seq, head, dk] (natural layout, +ones col)
            for nch in range(NCH):
                x_sb = stage.tile([128, DT_IN, 512], F16, tag="xT")
                nc.sync.dma_start(out=x_sb[:], in_=vT[:, :, nch * 512 : (nch + 1) * 512])
                if nch % 2 == 0:
                    mc = nch * 4
                    nc.sync.dma_start(
                        out=maskT_sb[:, mc : mc + 8, :], in_=maskT[:, mc : mc + 8, :]
                    )
                if nch == NCH - 1:
                    nc.sync.dma_start(out=wo_sb[:], in_=wo[:])

                for rt in range(4):
                    kt = nch * 4 + rt
                    ps = pv_pool.tile([128, CH], F32, tag="pv")
                    for ka in range(DT_IN):
                        nc.tensor.matmul(
                            ps[:],
                            x_sb[:, ka, rt * 128 : (rt + 1) * 128],
                            wv_sb[:, ka, :],
                            start=(ka == 0),
                            stop=(ka == DT_IN - 1),
                        )
                    # evacuate on ACT (idle during projections; DVE does K/Q)
                    nc.scalar.copy(
                        out=V_sb[:, kt, :, 0:DK],
                        in_=ps[:].rearrange("p (h e) -> p h e", e=DK),
                    )

        # ---- attention: 3 head pairs x 2 query columns ----
        # Per pair, the even head lives on PE rows 0-63 / the odd head on
        # 64-127 of channel tile p, so their K=64 score matmuls go to
        # different PE row-groups and run concurrently.
        with tc.tile_pool(name="pattn", bufs=1, space="PSUM") as pattn:
            for p in range(NPAIR):
                for qc in range(QC):
                    qs = qc * 512
                    ctx = pattn.tile([128, 2, 512], F32, tag="ctx")
                    for kt in range(KT_TILES):
                        sc = pattn.tile([128, 2, 512], F32, tag=f"sc{kt % 2}")
                        pr = probs_pool.tile([128, 2, 512], F16, tag="pr")
                        mk = probs_pool.tile([128, 2, 512], F16, tag="mk")
                        for h01 in range(2):
                            po = 64 * h01
                            nc.tensor.matmul(
                                sc[:, h01, :],
                                KT_sb[po : po + 64, p, kt * 128 : (kt + 1) * 128],
                                QT_sb[po : po + 64, p, qs : qs + 512],
                                start=True,
                                stop=True,
                            )
                        # exp straight from PSUM; 1/sqrt(dk)=1/8 scale and a
                        # softmax-invariant -3 bias folded into the ACT affine
                        nc.scalar.activation(
                            out=pr[:],
                            in_=sc[:],
                            func=mybir.ActivationFunctionType.Exp,
                            bias=expbias[:],
                            scale=0.125,
                        )
                        # multiplicative {0,1} mask on fp16 probs (DVE 2x)
                        for h01 in range(2):
                            nc.vector.tensor_mul(
                                out=mk[:, h01, :],
                                in0=pr[:, h01, :],
                                in1=maskT_sb[:, kt, qs : qs + 512],
                            )
                        for h01 in range(2):
                            nc.tensor.matmul(
                                ctx[0 : DK + 1, h01, :],
                                V_sb[:, kt, 2 * p + h01, :],
                                mk[:, h01, :],
                                start=(kt == 0),
                                stop=(kt == KT_TILES - 1),
                                skip_group_check=True,
                            )
                    # normalize: rows 0..63 are ctx^T, row 64 the denominator
                    den = small.tile([1, 1024], F32, tag="den")
                    nc.vector.tensor_copy(out=den[:], in_=ctx[DK : DK + 1, :, :])
                    recip = small.tile([1, 1024], F32, tag="recip")
                    nc.vector.reciprocal_approx_fast(out=recip[:], in_=den[:])
                    rep = small.tile([DK, 1024], F32, tag="rep")
                    nc.gpsimd.partition_broadcast(rep[:], recip[:])
                    for h01 in range(2):
                        nc.vector.tensor_mul(
                            out=ctx_sb[64 * h01 : 64 * h01 + 64, p, qs : qs + 512],
                            in0=ctx[0:DK, h01, :],
                            in1=rep[:, h01 * 512 : (h01 + 1) * 512],
                        )

        # ---- output projection (partial: this core's 6 heads) ----
        with tc.tile_pool(name="pout", bufs=2, space="PSUM") as pout:
            for d in range(DT_IN):
                for qc in range(QC):
                    ps = pout.tile([128, 512], F32, tag="po")
                    for ka in range(DT_CH):
                        nc.tensor.matmul(
                            ps[:],
                            wo_sb[:, ka, d * 128 : (d + 1) * 128],
                            ctx_sb[:, ka, qc * 512 : (qc + 1) * 512],
                            start=(ka == 0),
                            stop=(ka == DT_CH - 1),
                        )
                    o_sb = osb_pool.tile([128, 512], F16, tag="osb")
                    nc.vector.tensor_scalar_add(
                        out=o_sb[:],
                        in0=ps[:],
                        scalar1=bo_sb[:, d : d + 1],
                    )
                    nc.sync.dma_start(
                        out=outT[d * 128 : (d + 1) * 128, qc * 512 : (qc + 1) * 512],
                        in_=o_sb[:],
                    )


def _tile_dm(x):
    """[D, N] -> [128, D//128, N] fp16 (partition-tiled over the first dim)."""
    d, n = x.shape
    return np.ascontiguousarray(
        x.reshape(d // 128, 128, n).swapaxes(0, 1).astype(np.float16)
    )


def _tile_bias(b):
    """[C] -> [128, C//128] fp32."""
    c = b.shape[0]
    return np.ascontiguousarray(np.asarray(b, np.float32).reshape(c // 128, 128).T)


def _prep_inputs(q, k, v, mask, wq, bq, wk, bk, wv, bv, wo, bo):
    q = np.asarray(q, dtype=np.float32).reshape(S, D)
    k = np.asarray(k, dtype=np.float32).reshape(S, D)
    v = np.asarray(v, dtype=np.float32).reshape(S, D)
    mask = np.asarray(mask).reshape(S, S)
    wq, wk, wv, wo = (np.asarray(w, np.float32) for w in (wq, wk, wv, wo))
    bq, bk, bv, bo = (np.asarray(b, np.float32) for b in (bq, bk, bv, bo))

    kT_t = _tile_dm(k.T)  # [128, 6, 4096]
    vT_t = _tile_dm(v.T)

    in_maps = []
    for c in range(NCORES):
        qi, hi = c // NHG, c % NHG
        hs = slice(hi * CH, (hi + 1) * CH)
        qrows = slice(qi * SQ, (qi + 1) * SQ)
        # bv's contribution to ctx is den*bv^T which normalizes to +bv^T per
        # head; through wo that is the constant row bv_hg @ wo_hg.  bo is
        # applied once (head-group 0 only).
        bo_eff = bv[hs] @ wo[hs.start : hs.stop, :] + (bo if hi == 0 else 0.0)
        m = {
            "qT": _tile_dm(q[qrows, :].T),
            "kT": kT_t,
            "vT": vT_t,
            "maskT": np.ascontiguousarray(
                mask[qrows, :].T.reshape(KT_TILES, 128, SQ).swapaxes(0, 1)
            ).astype(np.float16),
            "wq": _tile_dm(wq[:, hs]),
            "wk": _tile_dm(wk[:, hs]),
            "wv": _tile_dm(wv[:, hs]),
            "wo": _tile_dm(wo[hs, :]),
            "bq": _tile_bias(bq[hs]),
            "bk": _tile_bias(bk[hs]),
            "bo": _tile_bias(bo_eff),
        }
        in_maps.append(m)
    return in_maps


def kernel(**inputs) -> np.ndarray:
    if "nc" not in _CACHE:
        _CACHE["nc"] = build_kernel()
    nc = _CACHE["nc"]
    in_maps = _prep_inputs(**inputs)
    res = bass_utils.run_bass_kernel_spmd(nc, in_maps, core_ids=list(range(NCORES)))
    parts = [np.asarray(res.results[c]["outT"], np.float32) for c in range(NCORES)]
    out = np.concatenate(
        [(parts[2 * qi] + parts[2 * qi + 1]).T for qi in range(NQSH)], axis=0
    )
    return out.reshape(B, S, D)


# revision 24
# speedup vs baseline: 1.2074x; 1.2074x over previous
"""Multi-head attention (B=1, S=4096, D=768, H=12) on 8 trn2 NeuronCores.

Sharding: 4 query shards x 2 head groups (6 heads each). Each core:
  - projects its 1024 q rows to QT [384, 1024] for its 6 heads
  - projects the full k, v to KT [384, 4096] / V_aug [4096, 6, 65]
    for its 6 heads (the 65th V column is ones so the probs@V matmul
    also produces the softmax denominator)
  - scores sT = K_h @ Q_h^T per head/k-tile on PE (K=64 contraction;
    the two heads of a pair sit on PE row-groups 0-63 / 64-127 so their
    score matmuls run concurrently)
  - exp on ACT directly from PSUM (scale=1/8, bias=-3 folded in; the -3
    is softmax-invariant and buys fp16 range), probs fp16 to SBUF
  - mask applied MULTIPLICATIVELY to probs on DVE (fp16 tensor_tensor
    at 2x rate) -- no fp32 PSUM pre-pass, which saturated DVE in v1
  - probs@V_aug accumulated on PE, normalization at the end
  - partial output projection with its 6 heads' wo slice; host adds the
    two head-group partials.  bv/bo are folded into a host-computed
    output bias (bv contribution = den * bv^T, and den normalizes to 1,
    so bo' = bv_hg @ wo_hg + bo).  bk is applied in the K projection.

Inputs are pre-transposed/pre-tiled/cast to fp16 on the host (layout
prep is part of sharding); all matmul accumulation is fp32 in PSUM.
"""

import numpy as np

import concourse.bass as bass
import concourse.mybir as mybir
import concourse.tile as tile
from concourse import bacc, bass_utils

B, S, D, H = 1, 4096, 768, 12
DK = D // H  # 64
NCORES = 8
NQSH = 4  # query shards
NHG = 2  # head groups
SQ = S // NQSH  # 1024 query rows per core
HL = H // NHG  # 6 local heads
NPAIR = HL // 2  # 3 local head pairs
CH = HL * DK  # 384 local channels
DT_IN = D // 128  # 6 input-dim tiles
DT_CH = CH // 128  # 3 channel tiles
KT_TILES = S // 128  # 32 k tiles
NCH = S // 512  # 8 column chunks for full-seq K/V projections
QC = SQ // 512  # 2 query column chunks

F16 = mybir.dt.float16
F32 = mybir.dt.float32

_CACHE = {}


def build_kernel(timing=False):
    nc = bacc.Bacc("TRN2", target_bir_lowering=False, debug=False, num_devices=NCORES)

    # timing=True: declare the big inputs as Internal DRAM (garbage contents,
    # identical DMA traffic) so per-exec host->device input copies don't
    # dominate the wall-clock slope measurement.
    kw = {} if timing else {"kind": "ExternalInput"}
    qT = nc.dram_tensor("qT", [128, DT_IN, SQ], F16, **kw)
    kT = nc.dram_tensor("kT", [128, DT_IN, S], F16, **kw)
    vT = nc.dram_tensor("vT", [128, DT_IN, S], F16, **kw)
    maskT = nc.dram_tensor("maskT", [128, KT_TILES, SQ], F16, **kw)
    wq = nc.dram_tensor("wq", [128, DT_IN, CH], F16, **kw)
    wk = nc.dram_tensor("wk", [128, DT_IN, CH], F16, **kw)
    wv = nc.dram_tensor("wv", [128, DT_IN, CH], F16, **kw)
    wo = nc.dram_tensor("wo", [128, DT_CH, D], F16, **kw)
    bq = nc.dram_tensor("bq", [128, DT_CH], F32, **kw)
    bk = nc.dram_tensor("bk", [128, DT_CH], F32, **kw)
    bo = nc.dram_tensor("bo", [128, DT_IN], F32, **kw)
    if timing:
        nc.dram_tensor("tinput", [1, 8], F32, kind="ExternalInput")
    outT = nc.dram_tensor("outT", [D, SQ], F16, kind="ExternalOutput")

    with tile.TileContext(nc) as tc:
        _build_tile(tc, qT, kT, vT, maskT, wq, wk, wv, wo, bq, bk, bo, outT)
    nc.compile()
    return nc


def _build_tile(tc, qT, kT, vT, maskT, wq, wk, wv, wo, bq, bk, bo, outT):
    nc = tc.nc

    with (
        tc.tile_pool(name="persist", bufs=1) as persist,
        tc.tile_pool(name="stage", bufs=3) as stage,
        tc.tile_pool(name="probs", bufs=2) as probs_pool,
        tc.tile_pool(name="small", bufs=1) as small,
        tc.tile_pool(name="osb", bufs=2) as osb_pool,
    ):
        # ---- persistent SBUF tensors ----
        wq_sb = persist.tile([128, DT_IN, CH], F16, name="wq_sb")
        nc.sync.dma_start(out=wq_sb[:], in_=wq[:])
        q_in = persist.tile([128, DT_IN, SQ], F16, name="q_in")
        nc.sync.dma_start(out=q_in[:], in_=qT[:])
        wk_sb = persist.tile([128, DT_IN, CH], F16, name="wk_sb")
        nc.sync.dma_start(out=wk_sb[:], in_=wk[:])
        wv_sb = persist.tile([128, DT_IN, CH], F16, name="wv_sb")
        nc.sync.dma_start(out=wv_sb[:], in_=wv[:])
        wo_sb = persist.tile([128, DT_CH, D], F16, name="wo_sb")
        nc.sync.dma_start(out=wo_sb[:], in_=wo[:])

        bq_sb = persist.tile([128, DT_CH], F32, name="bq_sb")
        nc.sync.dma_start(out=bq_sb[:], in_=bq[:])
        bk_sb = persist.tile([128, DT_CH], F32, name="bk_sb")
        nc.sync.dma_start(out=bk_sb[:], in_=bk[:])
        bo_sb = persist.tile([128, DT_IN], F32, name="bo_sb")
        nc.sync.dma_start(out=bo_sb[:], in_=bo[:])

        maskT_sb = persist.tile([128, KT_TILES, SQ], F16, name="maskT_sb")
        nc.sync.dma_start(out=maskT_sb[:], in_=maskT[:])

        KT_sb = persist.tile([128, DT_CH, S], F16, name="KT_sb")
        V_sb = persist.tile([128, KT_TILES, HL, DK + 1], F16, name="V_sb")
        QT_sb = persist.tile([128, DT_CH, SQ], F16, name="QT_sb")
        ctx_sb = persist.tile([128, DT_CH, SQ], F16, name="ctx_sb")

        # ones column for the V augmentation (denominator trick)
        nc.vector.memset(V_sb[:, :, :, DK : DK + 1], 1.0)

        # per-partition bias AP for exp's softmax-invariant -3 shift
        expbias = persist.tile([128, 1], F32, name="expbias")
        nc.vector.memset(expbias[:], -3.0)

        # ---- projections ----
        with (
            tc.tile_pool(name="pj", bufs=2, space="PSUM") as pj_pool,
            tc.tile_pool(name="pvp", bufs=2, space="PSUM") as pv_pool,
        ):
            # Q projection -> QT_sb [chan, q]
            for ch in range(DT_CH):
                for qc in range(QC):
                    ps = pj_pool.tile([128, 512], F32, tag="pj")
                    for ka in range(DT_IN):
                        nc.tensor.matmul(
                            ps[:],
                            wq_sb[:, ka, ch * 128 : (ch + 1) * 128],
                            q_in[:, ka, qc * 512 : (qc + 1) * 512],
                            start=(ka == 0),
                            stop=(ka == DT_IN - 1),
                        )
                    nc.vector.tensor_scalar_add(
                        out=QT_sb[:, ch, qc * 512 : (qc + 1) * 512],
                        in0=ps[:],
                        scalar1=bq_sb[:, ch : ch + 1],
                    )

            # K projection -> KT_sb [chan, seq]
            for nch in range(NCH):
                x_sb = stage.tile([128, DT_IN, 512], F16, tag="xT")
                nc.sync.dma_start(out=x_sb[:], in_=kT[:, :, nch * 512 : (nch + 1) * 512])
                for ch in range(DT_CH):
                    ps = pj_pool.tile([128, 512], F32, tag="pj")
                    for ka in range(DT_IN):
                        nc.tensor.matmul(
                            ps[:],
                            wk_sb[:, ka, ch * 128 : (ch + 1) * 128],
                            x_sb[:, ka, :],
                            start=(ka == 0),
                            stop=(ka == DT_IN - 1),
                        )
                    nc.vector.tensor_scalar_add(
                        out=KT_sb[:, ch, nch * 512 : (nch + 1) * 512],
                        in0=ps[:],
                        scalar1=bk_sb[:, ch : ch + 1],
                    )

            # V projection -> V_sb [seq, head, dk] (natural layout, +ones col)
            for nch in range(NCH):
                x_sb = stage.tile([128, DT_IN, 512], F16, tag="xT")
                nc.sync.dma_start(out=x_sb[:], in_=vT[:, :, nch * 512 : (nch + 1) * 512])

                for rt in range(4):
                    kt = nch * 4 + rt
                    ps = pv_pool.tile([128, CH], F32, tag="pv")
                    for ka in range(DT_IN):
                        nc.tensor.matmul(
                            ps[:],
                            x_sb[:, ka, rt * 128 : (rt + 1) * 128],
                            wv_sb[:, ka, :],
                            start=(ka == 0),
                            stop=(ka == DT_IN - 1),
                        )
                    # evacuate on ACT (idle during projections; DVE does K/Q)
                    nc.scalar.copy(
                        out=V_sb[:, kt, :, 0:DK],
                        in_=ps[:].rearrange("p (h e) -> p h e", e=DK),
                    )

        # ---- attention: 3 head pairs x 2 query columns ----
        # Per pair, the even head lives on PE rows 0-63 / the odd head on
        # 64-127 of channel tile p, so their K=64 score matmuls go to
        # different PE row-groups and run concurrently.
        with tc.tile_pool(name="pattn", bufs=1, space="PSUM") as pattn:
            for p in range(NPAIR):
                for qc in range(QC):
                    qs = qc * 512
                    ctx = pattn.tile([128, 2, 512], F32, tag="ctx")
                    for kt in range(KT_TILES):
                        sc = pattn.tile([128, 2, 512], F32, tag=f"sc{kt % 2}")
                        pr = probs_pool.tile([128, 2, 512], F16, tag="pr")
                        mk = probs_pool.tile([128, 2, 512], F16, tag="mk")
                        for h01 in range(2):
                            po = 64 * h01
                            nc.tensor.matmul(
                                sc[:, h01, :],
                                KT_sb[po : po + 64, p, kt * 128 : (kt + 1) * 128],
                                QT_sb[po : po + 64, p, qs : qs + 512],
                                start=True,
                                stop=True,
                            )
                        # exp straight from PSUM; 1/sqrt(dk)=1/8 scale and a
                        # softmax-invariant -3 bias folded into the ACT affine
                        nc.scalar.activation(
                            out=pr[:],
                            in_=sc[:],
                            func=mybir.ActivationFunctionType.Exp,
                            bias=expbias[:],
                            scale=0.125,
                        )
                        # multiplicative {0,1} mask on fp16 probs (DVE 2x)
                        for h01 in range(2):
                            nc.vector.tensor_mul(
                                out=mk[:, h01, :],
                                in0=pr[:, h01, :],
                                in1=maskT_sb[:, kt, qs : qs + 512],
                            )
                        for h01 in range(2):
                            nc.tensor.matmul(
                                ctx[0 : DK + 1, h01, :],
                                V_sb[:, kt, 2 * p + h01, :],
                                mk[:, h01, :],
                                start=(kt == 0),
                                stop=(kt == KT_TILES - 1),
                                skip_group_check=True,
                            )
                    # normalize: rows 0..63 are ctx^T, row 64 the denominator
                    den = small.tile([1, 1024], F32, tag="den")
                    nc.vector.tensor_copy(out=den[:], in_=ctx[DK : DK + 1, :, :])
                    recip = small.tile([1, 1024], F32, tag="recip")
                    nc.vector.reciprocal_approx_fast(out=recip[:], in_=den[:])
                    rep = small.tile([DK, 1024], F32, tag="rep")
                    nc.gpsimd.partition_broadcast(rep[:], recip[:])
                    for h01 in range(2):
                        nc.vector.tensor_mul(
                            out=ctx_sb[64 * h01 : 64 * h01 + 64, p, qs : qs + 512],
                            in0=ctx[0:DK, h01, :],
                            in1=rep[:, h01 * 512 : (h01 + 1) * 512],
                        )

        # ---- output projection (partial: this core's 6 heads) ----
        with tc.tile_pool(name="pout", bufs=2, space="PSUM") as pout:
            for d in range(DT_IN):
                for qc in range(QC):
                    ps = pout.tile([128, 512], F32, tag="po")
                    for ka in range(DT_CH):
                        nc.tensor.matmul(
                            ps[:],
                            wo_sb[:, ka, d * 128 : (d + 1) * 128],
                            ctx_sb[:, ka, qc * 512 : (qc + 1) * 512],
                            start=(ka == 0),
                            stop=(ka == DT_CH - 1),
                        )
                    o_sb = osb_pool.tile([128, 512], F16, tag="osb")
                    nc.vector.tensor_scalar_add(
                        out=o_sb[:],
                        in0=ps[:],
                        scalar1=bo_sb[:, d : d + 1],
                    )
                    nc.sync.dma_start(
                        out=outT[d * 128 : (d + 1) * 128, qc * 512 : (qc + 1) * 512],
                        in_=o_sb[:],
                    )


def _tile_dm(x):
    """[D, N] -> [128, D//128, N] fp16 (partition-tiled over the first dim)."""
    d, n = x.shape
    return np.ascontiguousarray(
        x.reshape(d // 128, 128, n).swapaxes(0, 1).astype(np.float16)
    )


def _tile_bias(b):
    """[C] -> [128, C//128] fp32."""
    c = b.shape[0]
    return np.ascontiguousarray(np.asarray(b, np.float32).reshape(c // 128, 128).T)


def _prep_inputs(q, k, v, mask, wq, bq, wk, bk, wv, bv, wo, bo):
    q = np.asarray(q, dtype=np.float32).reshape(S, D)
    k = np.asarray(k, dtype=np.float32).reshape(S, D)
    v = np.asarray(v, dtype=np.float32).reshape(S, D)
    mask = np.asarray(mask).reshape(S, S)
    wq, wk, wv, wo = (np.asarray(w, np.float32) for w in (wq, wk, wv, wo))
    bq, bk, bv, bo = (np.asarray(b, np.float32) for b in (bq, bk, bv, bo))

    kT_t = _tile_dm(k.T)  # [128, 6, 4096]
    vT_t = _tile_dm(v.T)

    in_maps = []
    for c in range(NCORES):
        qi, hi = c // NHG, c % NHG
        hs = slice(hi * CH, (hi + 1) * CH)
        qrows = slice(qi * SQ, (qi + 1) * SQ)
        # bv's contribution to ctx is den*bv^T which normalizes to +bv^T per
        # head; through wo that is the constant row bv_hg @ wo_hg.  bo is
        # applied once (head-group 0 only).
        bo_eff = bv[hs] @ wo[hs.start : hs.stop, :] + (bo if hi == 0 else 0.0)
        m = {
            "qT": _tile_dm(q[qrows, :].T),
            "kT": kT_t,
            "vT": vT_t,
            "maskT": np.ascontiguousarray(
                mask[qrows, :].T.reshape(KT_TILES, 128, SQ).swapaxes(0, 1)
            ).astype(np.float16),
            "wq": _tile_dm(wq[:, hs]),
            "wk": _tile_dm(wk[:, hs]),
            "wv": _tile_dm(wv[:, hs]),
            "wo": _tile_dm(wo[hs, :]),
            "bq": _tile_bias(bq[hs]),
            "bk": _tile_bias(bk[hs]),
            "bo": _tile_bias(bo_eff),
        }
        in_maps.append(m)
    return in_maps


def kernel(**inputs) -> np.ndarray:
    if "nc" not in _CACHE:
        _CACHE["nc"] = build_kernel()
    nc = _CACHE["nc"]
    in_maps = _prep_inputs(**inputs)
    res = bass_utils.run_bass_kernel_spmd(nc, in_maps, core_ids=list(range(NCORES)))
    parts = [np.asarray(res.results[c]["outT"], np.float32) for c in range(NCORES)]
    out = np.concatenate(
        [(parts[2 * qi] + parts[2 * qi + 1]).T for qi in range(NQSH)], axis=0
    )
    return out.reshape(B, S, D)
